# revision 41
# baseline (speedup 1.0000x reference)
"""Single-head causal attention with RoPE on 8 Trainium2 NeuronCores.

Problem: x:(8,2048,1024), Wq/Wk/Wv:(1024,64) -> out:(8,2048,64)
  q = rope(x@Wq); k = rope(x@Wk); v = x@Wv
  out = softmax(causal(q k^T / sqrt(64))) @ v

Sharding: data-parallel over batch B=8, one batch element per core.

v2 schedule (vs v1): DMA ordered by first-use (xT chunk stream on sync
alone; consts + per-chunk cos/sin slices on scalar); krope partition
dup on the vector HWDGE queue (SWDGE was ~5us/copy); warmup trimmed to
8 MMs and ungated; diag score blocks paired 2-per-PSUM-tile so one exp
ACTIVATE covers both; projection/v-proj matmuls injected between
attention units so the ACT exp stream never starves while PE fills its
slack; output DRAM layout [128,chunk,4,H] so each partition writes 1KB
contiguous (v1 wrote 256B packets); host-side rearrange to (T,H).
"""

import numpy as np
import ml_dtypes

B, T, C, H = 8, 2048, 1024, 64
NCORES = 8
CHUNK = 512
NCHUNK = T // CHUNK  # 4
NSB = T // 128       # 16 s-blocks
NCB = C // 128       # 8 c-blocks

bf16 = ml_dtypes.bfloat16


# ---------------------------------------------------------------- host consts
def _build_consts():
    half = H // 2
    inv_freq = (1.0 / (10000.0 ** (np.arange(half, dtype=np.float32) / half))).astype(
        np.float32
    )
    t = np.arange(T, dtype=np.float32)
    freqs = t[:, None] * inv_freq[None, :]  # (T, half) fp32
    cos = np.repeat(np.cos(freqs), 2, axis=-1)  # (T, H)
    sin = np.repeat(np.sin(freqs), 2, axis=-1)
    cosT = np.ascontiguousarray(cos.T)  # (H, T)
    sinT = np.ascontiguousarray(sin.T)

    coscos = np.concatenate([cosT, cosT], axis=0).astype(bf16)  # (128, T)
    sinsin = np.concatenate([sinT, sinT], axis=0).astype(bf16)

    # rot = R @ q with rot[2i] = -q[2i+1], rot[2i+1] = q[2i]
    Rm = np.zeros((H, H), np.float32)
    for i in range(half):
        Rm[2 * i, 2 * i + 1] = -1.0
        Rm[2 * i + 1, 2 * i] = 1.0
    r2 = np.zeros((128, 128), np.float32)
    r2[0:H, 0:H] = Rm.T
    r2[H:128, H:128] = Rm.T
    r2 = r2.astype(bf16)

    sl = np.arange(128)
    trimask = (sl[:, None] <= sl[None, :]).astype(bf16)  # (128, 128)

    identb2 = np.concatenate([np.eye(H), np.eye(H)], axis=0).astype(bf16)  # (128, 64)
    ident_f32 = np.eye(H + 2, dtype=bf16)  # padded to 66 for alignment

    # selection matrix: krope0[j] = (selk^T @ qkrope)[j] = qkrope[64+j]
    selk = np.concatenate([np.zeros((H, H)), np.eye(H)], axis=0).astype(bf16)

    return coscos, sinsin, r2, trimask, identb2, ident_f32, selk


# ---------------------------------------------------------------- bass program
def _build_bass():
    import concourse.mybir as mybir
    import concourse.tile as tile
    from concourse import bacc
    from concourse.bass import ts

    BF = mybir.dt.bfloat16
    F32 = mybir.dt.float32
    Exp = mybir.ActivationFunctionType.Exp

    nc = bacc.Bacc(
        "TRN2",
        target_bir_lowering=False,
        debug=False,
        enable_asserts=False,
        num_devices=NCORES,
    )

    # xT prepacked on host to SBUF layout [128(p), chunk, cblk, 512]
    xT_d = nc.dram_tensor("xTp", [128, NCHUNK, NCB, CHUNK], BF, kind="ExternalInput")
    wqk_d = nc.dram_tensor("wqkp", [128, NCB, 128], BF, kind="ExternalInput")
    wv_d = nc.dram_tensor("wvp", [128, NCB, H], BF, kind="ExternalInput")
    r2_d = nc.dram_tensor("r2", [128, 128], BF, kind="ExternalInput")
    coscos_d = nc.dram_tensor("coscos", [128, T], BF, kind="ExternalInput")
    sinsin_d = nc.dram_tensor("sinsin", [128, T], BF, kind="ExternalInput")
    trimask_d = nc.dram_tensor("trimask", [128, 128], BF, kind="ExternalInput")
    identb_d = nc.dram_tensor("identb2", [128, H], BF, kind="ExternalInput")
    identf_d = nc.dram_tensor("identf", [H + 2, H + 2], BF, kind="ExternalInput")
    selk_d = nc.dram_tensor("selk", [128, H], BF, kind="ExternalInput")
    # [partition, chunk, j, h]: t = chunk*512 + j*128 + p; 1KB contiguous/part
    out_d = nc.dram_tensor("out", [128, NCHUNK, 4, H], F32, kind="ExternalOutput")

    with tile.TileContext(nc) as tc:
        with (
            tc.tile_pool(name="persist", bufs=1) as persist,
            tc.tile_pool(name="work", bufs=3) as work,
            tc.tile_pool(name="pexpp", bufs=8) as pexpp,
            tc.tile_pool(name="ps_scratch", bufs=2, space="PSUM") as ps_scratch,
            tc.tile_pool(name="ps_sc", bufs=2, space="PSUM") as ps_sc,
            tc.tile_pool(name="ps_out", bufs=2, space="PSUM") as ps_out,
        ):
            # ---- persistent SBUF tensors
            wqk_sb = persist.tile([128, NCB, 128], BF)
            wv_sb = persist.tile([128, NCB, H], BF)
            r2_sb = persist.tile([128, 128], BF)
            coscos_sb = persist.tile([128, T], BF)
            sinsin_sb = persist.tile([128, T], BF)
            trimask_sb = persist.tile([128, 128], BF)
            identb_sb = persist.tile([128, H], BF)
            identf_sb = persist.tile([H + 2, H + 2], BF)
            xT_sb = persist.tile([128, NCHUNK, NCB, CHUNK], BF)
            qkrope = persist.tile([128, T], BF)   # q' rows 0:64, k' rows 64:128
            krope0 = persist.tile([H, T], BF)     # k' copy at partitions 0:64
            vT_sb = persist.tile([128, NCHUNK // 2, CHUNK], BF)  # stacked pairs
            vnat = persist.tile([128, NSB, H + 1], BF)

            # ---- sync HWDGE queue: xT stream ONLY (plus wqk, needed first).
            # chunk 0 split in half so projections start earlier.
            nc.sync.dma_start(out=wqk_sb[:], in_=wqk_d.ap())
            nc.sync.dma_start(out=xT_sb[:, 0, 0:4], in_=xT_d.ap()[:, 0, 0:4])
            nc.sync.dma_start(out=xT_sb[:, 0, 4:8], in_=xT_d.ap()[:, 0, 4:8])
            for i in range(1, NCHUNK):
                nc.sync.dma_start(out=xT_sb[:, i], in_=xT_d.ap()[:, i])
            # ---- scalar HWDGE queue: consts ordered by first use; cos/sin
            # chunk-0 slices first, rest deferred (emitted mid-schedule)
            nc.scalar.dma_start(out=r2_sb[:], in_=r2_d.ap())
            nc.scalar.dma_start(out=coscos_sb[:, 0:CHUNK], in_=coscos_d.ap()[:, 0:CHUNK])
            nc.scalar.dma_start(out=sinsin_sb[:, 0:CHUNK], in_=sinsin_d.ap()[:, 0:CHUNK])
            nc.scalar.dma_start(out=trimask_sb[:], in_=trimask_d.ap())
            nc.scalar.dma_start(out=identf_sb[:], in_=identf_d.ap())
            selk_sb = persist.tile([128, H], BF)
            nc.scalar.dma_start(out=selk_sb[:], in_=selk_d.ap())
            nc.scalar.dma_start(out=wv_sb[:], in_=wv_d.ap())
            # ---- gpsimd (SWDGE) queue: small late-need const
            nc.gpsimd.dma_start(out=identb_sb[:], in_=identb_d.ap())

            # zwarm memset FIRST so warmup is not gated by the vnat memset
            zwarm = persist.tile([128, CHUNK], BF)
            nc.vector.memset(zwarm[:], 0.0)
            nc.vector.memset(vnat[:], 1.0)  # ones col (64); cols 0:64 overwritten

            # PE warmup: ~3.4us of junk matmuls so the HAM clock-gate opens
            # to 2.4GHz before the first real projection
            warm_ps = ps_sc.tile([128, 2, CHUNK], F32, tag="sc", name="warm")
            for w in range(16):
                nc.tensor.matmul(
                    warm_ps[:, 0, :],
                    zwarm[:, 0:128],
                    zwarm[:],
                    start=(w == 0),
                    stop=(w == 15),
                )

            def emit_proj(i):
                """qk projection + evict + rot + rope for chunk i."""
                tsl = ts(i, CHUNK)
                qk_ps = ps_scratch.tile([128, CHUNK], F32, tag="scr", name=f"qk{i}")
                for c in range(NCB):
                    nc.tensor.matmul(
                        qk_ps[:],
                        wqk_sb[:, c, :],
                        xT_sb[:, i, c, :],
                        start=(c == 0),
                        stop=(c == NCB - 1),
                    )
                qkS = work.tile([128, CHUNK], BF, tag="qkS", name=f"qkS{i}")
                nc.vector.tensor_copy(out=qkS[:], in_=qk_ps[:])
                rot_ps = ps_scratch.tile([128, CHUNK], F32, tag="scr", name=f"rot{i}")
                nc.tensor.matmul(rot_ps[:], r2_sb[:], qkS[:], start=True, stop=True)

                tmp1 = work.tile([128, CHUNK], BF, tag="tmp1", name=f"t1_{i}")
                nc.vector.tensor_mul(tmp1[:], qkS[:], coscos_sb[:, tsl])
                tmp2 = work.tile([128, CHUNK], BF, tag="tmp2", name=f"t2_{i}")
                nc.vector.tensor_mul(tmp2[:], rot_ps[:], sinsin_sb[:, tsl])
                nc.vector.tensor_add(qkrope[:, tsl], tmp1[:], tmp2[:])
                # k' partition copy for use as scores lhsT (gpsimd queue)
                nc.gpsimd.dma_start(out=krope0[:, tsl], in_=qkrope[H:128, tsl])

            def emit_vproj_solo(i):
                """v-projection for one chunk (column group by parity)."""
                g, par = i // 2, i % 2
                v_ps = ps_scratch.tile([128, CHUNK], F32, tag="scr", name=f"v{i}")
                lo = H * par
                for c in range(NCB):
                    nc.tensor.matmul(
                        v_ps[lo : lo + H, :],
                        wv_sb[:, c, :],
                        xT_sb[:, i, c, :],
                        start=(c == 0),
                        stop=(c == NCB - 1),
                    )
                nc.vector.tensor_copy(
                    out=vT_sb[lo : lo + H, g, :], in_=v_ps[lo : lo + H, :]
                )

            def emit_b_half(g, half_):
                """transpose 4 s-blocks (one chunk) of vT group g into vnat."""
                vn_ps = ps_out.tile(
                    [128, 4, H], BF, tag="out", name=f"vn{g}_{half_}"
                )
                base = H * half_
                for j in range(4):
                    nc.tensor.transpose(
                        vn_ps[:, j, :],
                        vT_sb[base : base + H, g, ts(j, 128)],
                        identb_sb[base : base + H, :],
                    )
                first = 8 * g + 4 * half_
                nc.vector.tensor_copy(
                    out=vnat[:, first : first + 4, 0:H], in_=vn_ps[:]
                )

            def emit_junk(n, tag):
                """junk matmuls: hold the HAM clock-gate open across a known
                DMA wait (a PE-idle gap > ~3.4us re-throttles to 1.2GHz)."""
                jw = ps_scratch.tile([128, CHUNK], F32, tag="scr", name=f"jk{tag}")
                for w in range(n):
                    nc.tensor.matmul(
                        jw[:],
                        zwarm[:, 0:128],
                        zwarm[:],
                        start=(w == 0),
                        stop=(w == n - 1),
                    )

            out_tiles = {}
            out_tiles = {}

            def c_open(i):
                out_ps = ps_out.tile([H + 1, CHUNK], F32, tag="out", name=f"o{i}")
                out_tiles[i] = out_ps

            def sc_pair(i, p):
                """scores for full s-blocks (2p, 2p+1) vs q chunk i."""
                sc2 = ps_sc.tile([128, 2, CHUNK], F32, tag="sc", name=f"s{i}_{p}")
                for h_ in range(2):
                    sb = 2 * p + h_
                    nc.tensor.matmul(
                        sc2[:, h_, :],
                        krope0[:, ts(sb, 128)],
                        qkrope[0:H, ts(i, CHUNK)],
                        start=True,
                        stop=True,
                    )
                return sc2

            def sc_diag2(i, t):
                """scores for diag s-blocks (4i+2t, 4i+2t+1), causal-trimmed."""
                sc2 = ps_sc.tile([128, 2, CHUNK], F32, tag="sc", name=f"sd{i}_{t}")
                for h_ in range(2):
                    j = 2 * t + h_
                    lo = 128 * j
                    nc.tensor.matmul(
                        sc2[:, h_, lo:CHUNK],
                        krope0[:, ts(4 * i + j, 128)],
                        qkrope[0:H, i * CHUNK + lo : (i + 1) * CHUNK],
                        start=True,
                        stop=True,
                    )
                return sc2

            def exp_pair(i, p, sc2):
                pexp2 = pexpp.tile(
                    [128, 2, CHUNK], BF, tag="pexp", name=f"p{i}_{p}"
                )
                nc.scalar.activation(out=pexp2[:], in_=sc2[:], func=Exp, scale=0.125)
                return pexp2

            def pv_pair(i, p, pexp2):
                out_ps = out_tiles[i]
                for h_ in range(2):
                    sb = 2 * p + h_
                    nc.tensor.matmul(
                        out_ps[:],
                        vnat[:, sb, :],
                        pexp2[:, h_, :],
                        start=(sb == 0),
                        stop=False,
                    )

            def exp_diag2(i, t, sc2):
                pexp2 = pexpp.tile(
                    [128, 2, CHUNK], BF, tag="pexp", name=f"pd{i}_{t}"
                )
                for h_ in range(2):
                    j = 2 * t + h_
                    lo = 128 * j
                    nc.scalar.activation(
                        out=pexp2[:, h_, lo:CHUNK],
                        in_=sc2[:, h_, lo:CHUNK],
                        func=Exp,
                        scale=0.125,
                    )
                    nc.vector.tensor_mul(
                        pexp2[:, h_, lo : lo + 128],
                        pexp2[:, h_, lo : lo + 128],
                        trimask_sb[:],
                    )
                return pexp2

            def pv_diag2(i, t, pexp2):
                nsb = 4 * i + 4
                out_ps = out_tiles[i]
                for h_ in range(2):
                    j = 2 * t + h_
                    sb = 4 * i + j
                    lo = 128 * j
                    nc.tensor.matmul(
                        out_ps[:, lo:CHUNK],
                        vnat[:, sb, :],
                        pexp2[:, h_, lo:CHUNK],
                        start=(sb == 0),
                        stop=(sb == nsb - 1),
                    )

            outS_tiles = {}

            def n_evict(i):
                """evict out accumulator to SBUF, freeing the o-pool buf."""
                outS = work.tile([H + 1, CHUNK], BF, tag="outS", name=f"oS{i}")
                nc.vector.tensor_copy(out=outS[:], in_=out_tiles[i][:])
                outS_tiles[i] = outS

            def n_rest(i):
                """normalize via PE transpose + reciprocal, then DMA out."""
                outS = outS_tiles[i]
                tr_ps = ps_out.tile([128, 4, H + 2], BF, tag="out", name=f"tr{i}")
                for j in range(4):
                    nc.tensor.transpose(
                        tr_ps[:, j, 0 : H + 1],
                        outS[:, ts(j, 128)],
                        identf_sb[0 : H + 1, 0 : H + 1],
                    )
                recip4 = work.tile([128, 4], F32, tag="recip", name=f"r{i}")
                nc.vector.reciprocal(out=recip4[:], in_=tr_ps[:, :, H])
                out_sb = work.tile([128, 4, H], F32, tag="outN", name=f"oN{i}")
                import concourse.bass as _b

                recip_bc = _b.AP(
                    tensor=recip4.tensor,
                    offset=recip4.offset,
                    ap=[list(recip4.ap[0]), list(recip4.ap[1]), [0, H]],
                )
                nc.vector.tensor_mul(out_sb[:], tr_ps[:, :, 0:H], recip_bc)
                nc.sync.dma_start(out=out_d.ap()[:, i], in_=out_sb[:])

            # dummy ACTIVATE: pulls the ~1.3us exp table load into the
            # DMA-wait window at kernel start
            dummy = work.tile([128, 2], BF, tag="dum", name="dum")
            nc.scalar.activation(out=dummy[:], in_=zwarm[:, 0:2], func=Exp)

            # ------------- counter-safe emission schedule -------------
            # Monotonic-counter rule: a PE instruction emitted before an exp
            # only stalls it if its own gate resolves later than the exp's
            # natural time. So: DMA-gated projections go AFTER the current
            # chunk's exps; pv matmuls (gated on earlier ACT items) are
            # safe anywhere after. Junk bridges cover chunk-DMA waits.
            emit_proj(0)
            for i_ in range(1, NCHUNK):
                tsl_ = ts(i_, CHUNK)
                nc.scalar.dma_start(
                    out=coscos_sb[:, tsl_], in_=coscos_d.ap()[:, tsl_]
                )
                nc.scalar.dma_start(
                    out=sinsin_sb[:, tsl_], in_=sinsin_d.ap()[:, tsl_]
                )
            c_open(0)
            s0 = sc_diag2(0, 0)
            s1 = sc_diag2(0, 1)
            emit_vproj_solo(0)
            emit_b_half(0, 0)
            e0 = exp_diag2(0, 0, s0)
            e1 = exp_diag2(0, 1, s1)
            emit_junk(12, "a")
            emit_proj(1)
            pv_diag2(0, 0, e0)
            pv_diag2(0, 1, e1)
            emit_vproj_solo(1)
            emit_b_half(0, 1)
            n_evict(0)
            # ---- C1
            c_open(1)
            t0 = sc_pair(1, 0)
            t1 = sc_pair(1, 1)
            ep0 = exp_pair(1, 0, t0)
            d0 = sc_diag2(1, 0)
            ep1 = exp_pair(1, 1, t1)
            d1 = sc_diag2(1, 1)
            ed0 = exp_diag2(1, 0, d0)
            ed1 = exp_diag2(1, 1, d1)
            pv_pair(1, 0, ep0)
            pv_pair(1, 1, ep1)
            emit_junk(4, "b")
            emit_proj(2)
            pv_diag2(1, 0, ed0)
            pv_diag2(1, 1, ed1)
            emit_vproj_solo(2)
            emit_b_half(1, 0)
            n_evict(1)
            n_rest(0)
            # ---- C2
            c_open(2)
            t0 = sc_pair(2, 0)
            t1 = sc_pair(2, 1)
            ep0 = exp_pair(2, 0, t0)
            t2 = sc_pair(2, 2)
            ep1 = exp_pair(2, 1, t1)
            t3 = sc_pair(2, 3)
            ep2 = exp_pair(2, 2, t2)
            d0 = sc_diag2(2, 0)
            ep3 = exp_pair(2, 3, t3)
            d1 = sc_diag2(2, 1)
            ed0 = exp_diag2(2, 0, d0)
            ed1 = exp_diag2(2, 1, d1)
            pv_pair(2, 0, ep0)
            pv_pair(2, 1, ep1)
            pv_pair(2, 2, ep2)
            pv_pair(2, 3, ep3)
            emit_junk(4, "c")
            emit_proj(3)
            pv_diag2(2, 0, ed0)
            pv_diag2(2, 1, ed1)
            emit_vproj_solo(3)
            emit_b_half(1, 1)
            n_evict(2)
            n_rest(1)
            # ---- C3
            c_open(3)
            t0 = sc_pair(3, 0)
            t1 = sc_pair(3, 1)
            ep0 = exp_pair(3, 0, t0)
            t2 = sc_pair(3, 2)
            ep1 = exp_pair(3, 1, t1)
            t3 = sc_pair(3, 3)
            ep2 = exp_pair(3, 2, t2)
            t4 = sc_pair(3, 4)
            ep3 = exp_pair(3, 3, t3)
            t5 = sc_pair(3, 5)
            ep4 = exp_pair(3, 4, t4)
            d0 = sc_diag2(3, 0)
            ep5 = exp_pair(3, 5, t5)
            d1 = sc_diag2(3, 1)
            ed0 = exp_diag2(3, 0, d0)
            ed1 = exp_diag2(3, 1, d1)
            pv_pair(3, 0, ep0)
            pv_pair(3, 1, ep1)
            pv_pair(3, 2, ep2)
            pv_pair(3, 3, ep3)
            n_rest(2)
            pv_pair(3, 4, ep4)
            pv_pair(3, 5, ep5)
            pv_diag2(3, 0, ed0)
            pv_diag2(3, 1, ed1)
            n_evict(3)
            n_rest(3)

    nc.compile()
    return nc


_NC_CACHE = None


def _get_nc():
    global _NC_CACHE
    if _NC_CACHE is None:
        _NC_CACHE = _build_bass()
    return _NC_CACHE


def make_in_maps(x, Wq, Wk, Wv):
    """Host-side prep: shard over batch + precompute constants."""
    coscos, sinsin, r2, trimask, identb2, ident_f32, selk = _build_consts()
    wqk = np.concatenate([Wq, Wk], axis=1).astype(bf16)  # (C, 128)
    wv = Wv.astype(bf16)
    wqkp = np.ascontiguousarray(wqk.reshape(NCB, 128, 128).transpose(1, 0, 2))
    wvp = np.ascontiguousarray(wv.reshape(NCB, 128, H).transpose(1, 0, 2))
    in_maps = []
    for b in range(B):
        xT = x[b].T.astype(bf16)  # (C, T)
        xTp = np.ascontiguousarray(
            xT.reshape(NCB, 128, NCHUNK, CHUNK).transpose(1, 2, 0, 3)
        )
        in_maps.append(
            {
                "xTp": xTp,
                "wqkp": wqkp,
                "wvp": wvp,
                "r2": r2,
                "coscos": coscos,
                "sinsin": sinsin,
                "trimask": trimask,
                "identb2": identb2,
                "identf": ident_f32,
                "selk": selk,
            }
        )
    return in_maps


def kernel(x, Wq, Wk, Wv):
    from concourse.bass_utils import run_bass_kernel_spmd

    x = np.asarray(x, dtype=np.float32)
    Wq = np.asarray(Wq, dtype=np.float32)
    Wk = np.asarray(Wk, dtype=np.float32)
    Wv = np.asarray(Wv, dtype=np.float32)

    nc = _get_nc()
    in_maps = make_in_maps(x, Wq, Wk, Wv)
    res = run_bass_kernel_spmd(nc, in_maps, core_ids=list(range(NCORES)))
    # out buffer [128, chunk, j, h]: t = chunk*512 + j*128 + p
    out = np.stack([r["out"] for r in res.results])  # (B, 128, NCHUNK, 4, H)
    out = out.transpose(0, 2, 3, 1, 4).reshape(B, T, H)
    return np.ascontiguousarray(out.astype(np.float32))


# revision 42
# speedup vs baseline: 1.1987x; 1.1987x over previous
"""Single-head causal attention with RoPE on 8 Trainium2 NeuronCores.

Problem: x:(8,2048,1024), Wq/Wk/Wv:(1024,64) -> out:(8,2048,64)
  q = rope(x@Wq); k = rope(x@Wk); v = x@Wv
  out = softmax(causal(q k^T / sqrt(64))) @ v

Sharding: data-parallel over batch B=8, one batch element per core.

v2 schedule (vs v1): DMA ordered by first-use (xT chunk stream on sync
alone; consts + per-chunk cos/sin slices on scalar); krope partition
dup on the vector HWDGE queue (SWDGE was ~5us/copy); warmup trimmed to
8 MMs and ungated; diag score blocks paired 2-per-PSUM-tile so one exp
ACTIVATE covers both; projection/v-proj matmuls injected between
attention units so the ACT exp stream never starves while PE fills its
slack; output DRAM layout [128,chunk,4,H] so each partition writes 1KB
contiguous (v1 wrote 256B packets); host-side rearrange to (T,H).
"""

import numpy as np
import ml_dtypes

B, T, C, H = 8, 2048, 1024, 64
NCORES = 8
CHUNK = 512
NCHUNK = T // CHUNK  # 4
NSB = T // 128       # 16 s-blocks
NCB = C // 128       # 8 c-blocks

bf16 = ml_dtypes.bfloat16


# ---------------------------------------------------------------- host consts
def _build_consts():
    half = H // 2
    inv_freq = (1.0 / (10000.0 ** (np.arange(half, dtype=np.float32) / half))).astype(
        np.float32
    )
    t = np.arange(T, dtype=np.float32)
    freqs = t[:, None] * inv_freq[None, :]  # (T, half) fp32
    cos = np.repeat(np.cos(freqs), 2, axis=-1)  # (T, H)
    sin = np.repeat(np.sin(freqs), 2, axis=-1)
    cosT = np.ascontiguousarray(cos.T)  # (H, T)
    sinT = np.ascontiguousarray(sin.T)

    coscos = np.concatenate([cosT, cosT], axis=0).astype(bf16)  # (128, T)
    sinsin = np.concatenate([sinT, sinT], axis=0).astype(bf16)

    # rot = R @ q with rot[2i] = -q[2i+1], rot[2i+1] = q[2i]
    Rm = np.zeros((H, H), np.float32)
    for i in range(half):
        Rm[2 * i, 2 * i + 1] = -1.0
        Rm[2 * i + 1, 2 * i] = 1.0
    r2 = np.zeros((128, 128), np.float32)
    r2[0:H, 0:H] = Rm.T
    r2[H:128, H:128] = Rm.T
    r2 = r2.astype(bf16)

    sl = np.arange(128)
    trimask = (sl[:, None] <= sl[None, :]).astype(bf16)  # (128, 128)

    identb2 = np.concatenate([np.eye(H), np.eye(H)], axis=0).astype(bf16)  # (128, 64)
    ident_f32 = np.eye(H + 2, dtype=bf16)  # padded to 66 for alignment

    # selection matrix: krope0[j] = (selk^T @ qkrope)[j] = qkrope[64+j]
    selk = np.concatenate([np.zeros((H, H)), np.eye(H)], axis=0).astype(bf16)

    return coscos, sinsin, r2, trimask, identb2, ident_f32, selk


# ---------------------------------------------------------------- bass program
def _build_bass():
    import concourse.mybir as mybir
    import concourse.tile as tile
    from concourse import bacc
    from concourse.bass import ts

    BF = mybir.dt.bfloat16
    F32 = mybir.dt.float32
    Exp = mybir.ActivationFunctionType.Exp

    nc = bacc.Bacc(
        "TRN2",
        target_bir_lowering=False,
        debug=False,
        enable_asserts=False,
        num_devices=NCORES,
    )

    # xT prepacked on host to SBUF layout [128(p), chunk, cblk, 512]
    xT_d = nc.dram_tensor("xTp", [128, NCHUNK, NCB, CHUNK], BF, kind="ExternalInput")
    wqk_d = nc.dram_tensor("wqkp", [128, NCB, 128], BF, kind="ExternalInput")
    wv_d = nc.dram_tensor("wvp", [128, NCB, H], BF, kind="ExternalInput")
    r2_d = nc.dram_tensor("r2", [128, 128], BF, kind="ExternalInput")
    coscos_d = nc.dram_tensor("coscos", [128, T], BF, kind="ExternalInput")
    sinsin_d = nc.dram_tensor("sinsin", [128, T], BF, kind="ExternalInput")
    trimask_d = nc.dram_tensor("trimask", [128, 128], BF, kind="ExternalInput")
    identb_d = nc.dram_tensor("identb2", [128, H], BF, kind="ExternalInput")
    identf_d = nc.dram_tensor("identf", [H + 2, H + 2], BF, kind="ExternalInput")
    selk_d = nc.dram_tensor("selk", [128, H], BF, kind="ExternalInput")
    # [partition, chunk, j, h]: t = chunk*512 + j*128 + p; 1KB contiguous/part
    out_d = nc.dram_tensor("out", [128, NCHUNK, 4, H], F32, kind="ExternalOutput")

    with tile.TileContext(nc) as tc:
        with (
            tc.tile_pool(name="persist", bufs=1) as persist,
            tc.tile_pool(name="work", bufs=3) as work,
            tc.tile_pool(name="pexpp", bufs=5) as pexpp,
            tc.tile_pool(name="ps_scratch", bufs=2, space="PSUM") as ps_scratch,
            tc.tile_pool(name="ps_sc", bufs=2, space="PSUM") as ps_sc,
            tc.tile_pool(name="ps_out", bufs=2, space="PSUM") as ps_out,
        ):
            # ---- persistent SBUF tensors
            wqk_sb = persist.tile([128, NCB, 128], BF)
            wv_sb = persist.tile([128, NCB, H], BF)
            r2_sb = persist.tile([128, 128], BF)
            coscos_sb = persist.tile([128, T], BF)
            sinsin_sb = persist.tile([128, T], BF)
            trimask_sb = persist.tile([128, 128], BF)
            identb_sb = persist.tile([128, H], BF)
            identf_sb = persist.tile([H + 2, H + 2], BF)
            xT_sb = persist.tile([128, NCHUNK, NCB, CHUNK], BF)
            qkrope = persist.tile([128, T], BF)   # q' rows 0:64, k' rows 64:128
            krope0 = persist.tile([H, T], BF)     # k' copy at partitions 0:64
            vT_sb = persist.tile([128, NCHUNK // 2, CHUNK], BF)  # stacked pairs
            vnat = persist.tile([128, NSB, H + 1], BF)

            # ---- sync HWDGE queue: xT stream ONLY (plus wqk, needed first).
            # chunk 0 split in half so projections start earlier.
            nc.sync.dma_start(out=wqk_sb[:], in_=wqk_d.ap())
            nc.sync.dma_start(out=xT_sb[:, 0, 0:4], in_=xT_d.ap()[:, 0, 0:4])
            nc.sync.dma_start(out=xT_sb[:, 0, 4:8], in_=xT_d.ap()[:, 0, 4:8])
            for i in range(1, NCHUNK):
                nc.sync.dma_start(out=xT_sb[:, i], in_=xT_d.ap()[:, i])
            # ---- scalar HWDGE queue: consts ordered by first use; cos/sin
            # chunk-0 slices first, rest deferred (emitted mid-schedule)
            nc.scalar.dma_start(out=r2_sb[:], in_=r2_d.ap())
            nc.scalar.dma_start(out=coscos_sb[:, 0:CHUNK], in_=coscos_d.ap()[:, 0:CHUNK])
            nc.scalar.dma_start(out=sinsin_sb[:, 0:CHUNK], in_=sinsin_d.ap()[:, 0:CHUNK])
            nc.scalar.dma_start(out=trimask_sb[:], in_=trimask_d.ap())
            nc.scalar.dma_start(out=identf_sb[:], in_=identf_d.ap())
            selk_sb = persist.tile([128, H], BF)
            nc.scalar.dma_start(out=selk_sb[:], in_=selk_d.ap())
            nc.scalar.dma_start(out=wv_sb[:], in_=wv_d.ap())
            # ---- gpsimd (SWDGE) queue: small late-need const
            nc.gpsimd.dma_start(out=identb_sb[:], in_=identb_d.ap())

            # zwarm memset FIRST so warmup is not gated by the vnat memset
            zwarm = persist.tile([128, CHUNK], BF)
            nc.vector.memset(zwarm[:], 0.0)
            nc.vector.memset(vnat[:], 1.0)  # ones col (64); cols 0:64 overwritten

            # PE warmup: ~3.4us of junk matmuls so the HAM clock-gate opens
            # to 2.4GHz before the first real projection
            warm_ps = ps_sc.tile([128, 2, CHUNK], F32, tag="sc", name="warm")
            for w in range(8):
                nc.tensor.matmul(
                    warm_ps[:, 0, :],
                    zwarm[:, 0:128],
                    zwarm[:],
                    start=(w == 0),
                    stop=(w == 7),
                )

            def emit_proj(i):
                """qk projection + evict + rot + rope for chunk i."""
                tsl = ts(i, CHUNK)
                qk_ps = ps_scratch.tile([128, CHUNK], F32, tag="scr", name=f"qk{i}")
                for c in range(NCB):
                    nc.tensor.matmul(
                        qk_ps[:],
                        wqk_sb[:, c, :],
                        xT_sb[:, i, c, :],
                        start=(c == 0),
                        stop=(c == NCB - 1),
                    )
                qkS = work.tile([128, CHUNK], BF, tag="qkS", name=f"qkS{i}")
                nc.vector.tensor_copy(out=qkS[:], in_=qk_ps[:])
                rot_ps = ps_scratch.tile([128, CHUNK], F32, tag="scr", name=f"rot{i}")
                nc.tensor.matmul(rot_ps[:], r2_sb[:], qkS[:], start=True, stop=True)

                tmp1 = work.tile([128, CHUNK], BF, tag="tmp1", name=f"t1_{i}")
                nc.vector.tensor_mul(tmp1[:], qkS[:], coscos_sb[:, tsl])
                tmp2 = work.tile([128, CHUNK], BF, tag="tmp2", name=f"t2_{i}")
                nc.vector.tensor_mul(tmp2[:], rot_ps[:], sinsin_sb[:, tsl])
                nc.vector.tensor_add(qkrope[:, tsl], tmp1[:], tmp2[:])
                # k' partition copy for use as scores lhsT (gpsimd queue)
                nc.gpsimd.dma_start(out=krope0[:, tsl], in_=qkrope[H:128, tsl])

            def emit_vproj(g):
                """v-projection for chunk pair (2g, 2g+1), col-tiled so both
                column groups stream concurrently."""
                v_ps = ps_scratch.tile([128, CHUNK], F32, tag="scr", name=f"v{g}")
                for c in range(NCB):
                    nc.tensor.matmul(
                        v_ps[0:H, :],
                        wv_sb[:, c, :],
                        xT_sb[:, 2 * g, c, :],
                        start=(c == 0),
                        stop=(c == NCB - 1),
                        skip_group_check=True,
                    )
                    nc.tensor.matmul(
                        v_ps[H:128, :],
                        wv_sb[:, c, :],
                        xT_sb[:, 2 * g + 1, c, :],
                        start=(c == 0),
                        stop=(c == NCB - 1),
                        skip_group_check=True,
                    )
                nc.vector.tensor_copy(out=vT_sb[:, g, :], in_=v_ps[:])

            def emit_b(g):
                """transpose 8 s-blocks of the vT pair group g into vnat."""
                for half_ in range(2):
                    vn_ps = ps_out.tile(
                        [128, 4, H], BF, tag="out", name=f"vn{g}_{half_}"
                    )
                    base = H * half_
                    for j in range(4):
                        nc.tensor.transpose(
                            vn_ps[:, j, :],
                            vT_sb[base : base + H, g, ts(j, 128)],
                            identb_sb[base : base + H, :],
                        )
                    first = 8 * g + 4 * half_
                    nc.vector.tensor_copy(
                        out=vnat[:, first : first + 4, 0:H], in_=vn_ps[:]
                    )

            out_tiles = {}

            def c_open(i):
                out_ps = ps_out.tile([H + 1, CHUNK], F32, tag="out", name=f"o{i}")
                out_tiles[i] = out_ps

            def sc_pair(i, p):
                """scores for full s-blocks (2p, 2p+1) vs q chunk i."""
                sc2 = ps_sc.tile([128, 2, CHUNK], F32, tag="sc", name=f"s{i}_{p}")
                for h_ in range(2):
                    sb = 2 * p + h_
                    nc.tensor.matmul(
                        sc2[:, h_, :],
                        krope0[:, ts(sb, 128)],
                        qkrope[0:H, ts(i, CHUNK)],
                        start=True,
                        stop=True,
                    )
                return sc2

            def sc_diag2(i, t):
                """scores for diag s-blocks (4i+2t, 4i+2t+1), causal-trimmed."""
                sc2 = ps_sc.tile([128, 2, CHUNK], F32, tag="sc", name=f"sd{i}_{t}")
                for h_ in range(2):
                    j = 2 * t + h_
                    lo = 128 * j
                    nc.tensor.matmul(
                        sc2[:, h_, lo:CHUNK],
                        krope0[:, ts(4 * i + j, 128)],
                        qkrope[0:H, i * CHUNK + lo : (i + 1) * CHUNK],
                        start=True,
                        stop=True,
                    )
                return sc2

            def exp_pv_pair(i, p, sc2):
                nsb = 4 * i + 4
                out_ps = out_tiles[i]
                pexp2 = pexpp.tile(
                    [128, 2, CHUNK], BF, tag="pexp", name=f"p{i}_{p}"
                )
                nc.scalar.activation(out=pexp2[:], in_=sc2[:], func=Exp, scale=0.125)
                for h_ in range(2):
                    sb = 2 * p + h_
                    nc.tensor.matmul(
                        out_ps[:],
                        vnat[:, sb, :],
                        pexp2[:, h_, :],
                        start=(sb == 0),
                        stop=False,
                    )

            def exp_pv_diag2(i, t, sc2):
                nsb = 4 * i + 4
                out_ps = out_tiles[i]
                pexp2 = pexpp.tile(
                    [128, 2, CHUNK], BF, tag="pexp", name=f"pd{i}_{t}"
                )
                for h_ in range(2):
                    j = 2 * t + h_
                    sb = 4 * i + j
                    lo = 128 * j
                    nc.scalar.activation(
                        out=pexp2[:, h_, lo:CHUNK],
                        in_=sc2[:, h_, lo:CHUNK],
                        func=Exp,
                        scale=0.125,
                    )
                    nc.vector.tensor_mul(
                        pexp2[:, h_, lo : lo + 128],
                        pexp2[:, h_, lo : lo + 128],
                        trimask_sb[:],
                    )
                    nc.tensor.matmul(
                        out_ps[:, lo:CHUNK],
                        vnat[:, sb, :],
                        pexp2[:, h_, lo:CHUNK],
                        start=(sb == 0),
                        stop=(sb == nsb - 1),
                    )

            outS_tiles = {}

            def n_evict(i):
                """evict out accumulator to SBUF, freeing the o-pool buf."""
                outS = work.tile([H + 1, CHUNK], BF, tag="outS", name=f"oS{i}")
                nc.vector.tensor_copy(out=outS[:], in_=out_tiles[i][:])
                outS_tiles[i] = outS

            def n_rest(i):
                """normalize via PE transpose + reciprocal, then DMA out."""
                outS = outS_tiles[i]
                tr_ps = ps_out.tile([128, 4, H + 2], BF, tag="out", name=f"tr{i}")
                for j in range(4):
                    nc.tensor.transpose(
                        tr_ps[:, j, 0 : H + 1],
                        outS[:, ts(j, 128)],
                        identf_sb[0 : H + 1, 0 : H + 1],
                    )
                recip4 = work.tile([128, 4], F32, tag="recip", name=f"r{i}")
                nc.vector.reciprocal(out=recip4[:], in_=tr_ps[:, :, H])
                out_sb = work.tile([128, 4, H], F32, tag="outN", name=f"oN{i}")
                import concourse.bass as _b

                recip_bc = _b.AP(
                    tensor=recip4.tensor,
                    offset=recip4.offset,
                    ap=[list(recip4.ap[0]), list(recip4.ap[1]), [0, H]],
                )
                nc.vector.tensor_mul(out_sb[:], tr_ps[:, :, 0:H], recip_bc)
                nc.sync.dma_start(out=out_d.ap()[:, i], in_=out_sb[:])

            # dummy ACTIVATE: pulls the ~1.3us exp table load into the
            # DMA-wait window at kernel start
            dummy = work.tile([128, 2], BF, tag="dum", name="dum")
            nc.scalar.activation(out=dummy[:], in_=zwarm[:, 0:2], func=Exp)

            # ------------- interleaved emission schedule -------------
            # ACT is the metronome in phase C; proj/vproj matmuls slot into
            # the PE slack between attention units so neither engine starves.
            emit_proj(0)
            # deferred cos/sin slices: transfers complete before each needs
            for i_ in range(1, NCHUNK):
                tsl_ = ts(i_, CHUNK)
                nc.scalar.dma_start(
                    out=coscos_sb[:, tsl_], in_=coscos_d.ap()[:, tsl_]
                )
                nc.scalar.dma_start(
                    out=sinsin_sb[:, tsl_], in_=sinsin_d.ap()[:, tsl_]
                )
            c_open(0)
            s00 = sc_diag2(0, 0)
            s01 = sc_diag2(0, 1)
            emit_proj(1)
            emit_vproj(0)
            emit_b(0)
            exp_pv_diag2(0, 0, s00)
            exp_pv_diag2(0, 1, s01)
            n_evict(0)
            # ---- C1
            c_open(1)
            t0 = sc_pair(1, 0)
            t1 = sc_pair(1, 1)
            exp_pv_pair(1, 0, t0)
            d0 = sc_diag2(1, 0)
            exp_pv_pair(1, 1, t1)
            d1 = sc_diag2(1, 1)
            emit_proj(2)
            exp_pv_diag2(1, 0, d0)
            exp_pv_diag2(1, 1, d1)
            n_evict(1)
            n_rest(0)
            # ---- C2
            c_open(2)
            t0 = sc_pair(2, 0)
            t1 = sc_pair(2, 1)
            exp_pv_pair(2, 0, t0)
            t2 = sc_pair(2, 2)
            exp_pv_pair(2, 1, t1)
            t3 = sc_pair(2, 3)
            exp_pv_pair(2, 2, t2)
            d0 = sc_diag2(2, 0)
            exp_pv_pair(2, 3, t3)
            d1 = sc_diag2(2, 1)
            emit_proj(3)
            emit_vproj(1)
            emit_b(1)
            exp_pv_diag2(2, 0, d0)
            exp_pv_diag2(2, 1, d1)
            n_evict(2)
            n_rest(1)
            # ---- C3
            c_open(3)
            t0 = sc_pair(3, 0)
            t1 = sc_pair(3, 1)
            exp_pv_pair(3, 0, t0)
            t2 = sc_pair(3, 2)
            exp_pv_pair(3, 1, t1)
            t3 = sc_pair(3, 3)
            exp_pv_pair(3, 2, t2)
            t4 = sc_pair(3, 4)
            exp_pv_pair(3, 3, t3)
            t5 = sc_pair(3, 5)
            exp_pv_pair(3, 4, t4)
            d0 = sc_diag2(3, 0)
            exp_pv_pair(3, 5, t5)
            d1 = sc_diag2(3, 1)
            n_rest(2)
            exp_pv_diag2(3, 0, d0)
            exp_pv_diag2(3, 1, d1)
            n_evict(3)
            n_rest(3)

    nc.compile()
    return nc


_NC_CACHE = None


def _get_nc():
    global _NC_CACHE
    if _NC_CACHE is None:
        _NC_CACHE = _build_bass()
    return _NC_CACHE


def make_in_maps(x, Wq, Wk, Wv):
    """Host-side prep: shard over batch + precompute constants."""
    coscos, sinsin, r2, trimask, identb2, ident_f32, selk = _build_consts()
    wqk = np.concatenate([Wq, Wk], axis=1).astype(bf16)  # (C, 128)
    wv = Wv.astype(bf16)
    wqkp = np.ascontiguousarray(wqk.reshape(NCB, 128, 128).transpose(1, 0, 2))
    wvp = np.ascontiguousarray(wv.reshape(NCB, 128, H).transpose(1, 0, 2))
    in_maps = []
    for b in range(B):
        xT = x[b].T.astype(bf16)  # (C, T)
        xTp = np.ascontiguousarray(
            xT.reshape(NCB, 128, NCHUNK, CHUNK).transpose(1, 2, 0, 3)
        )
        in_maps.append(
            {
                "xTp": xTp,
                "wqkp": wqkp,
                "wvp": wvp,
                "r2": r2,
                "coscos": coscos,
                "sinsin": sinsin,
                "trimask": trimask,
                "identb2": identb2,
                "identf": ident_f32,
                "selk": selk,
            }
        )
    return in_maps


def kernel(x, Wq, Wk, Wv):
    from concourse.bass_utils import run_bass_kernel_spmd

    x = np.asarray(x, dtype=np.float32)
    Wq = np.asarray(Wq, dtype=np.float32)
    Wk = np.asarray(Wk, dtype=np.float32)
    Wv = np.asarray(Wv, dtype=np.float32)

    nc = _get_nc()
    in_maps = make_in_maps(x, Wq, Wk, Wv)
    res = run_bass_kernel_spmd(nc, in_maps, core_ids=list(range(NCORES)))
    # out buffer [128, chunk, j, h]: t = chunk*512 + j*128 + p
    out = np.stack([r["out"] for r in res.results])  # (B, 128, NCHUNK, 4, H)
    out = out.transpose(0, 2, 3, 1, 4).reshape(B, T, H)
    return np.ascontiguousarray(out.astype(np.float32))
